# revision 2
# baseline (speedup 1.0000x reference)
"""Trainium2 Bass kernel for nn_AudioClassifier (spiking CNN, LIF neurons).

Data-parallel over 8 NeuronCores: B=512 -> 64 per core. Per core, a
T=100 sequential scan; convs/FCs run on the PE as banded matmuls in a
feature-major layout [feature_partition, batch_free]; LIF updates run on
the vector engine; maxpool2 is a free-dim strided max (even/odd conv1
output positions are emitted into adjacent free-column blocks).
"""

import numpy as np

B, T, L = 512, 100, 686
NCORES = 8
BL = B // NCORES            # 64 samples per core
LP = 768                    # padded row length (6 windows of 128)
NW = 6                      # x windows per timestep
C1, K1 = 16, 13             # conv1: 16 ch, kernel 13, stride 5, pad 1
J1 = 136                    # conv1 out positions
C2, K2 = 32, 7              # conv2: stride 3, pad 1
J2 = 22                     # conv2 out positions
JP = 68                     # pooled positions
NM1 = 9                     # conv1 m-blocks (16 j each, last half)
NB1 = 2 * NM1               # 18 blocks of (8 j x 16 c); bi = 2m + (j%2)
NB2 = 6                     # conv2 output blocks (4 jj x 32 co)
BETA, THETA = 0.9, 1.0

_CACHE = {}


def _build_host_data(w1, b1, w2, b2, wf1, bf1, wf2, bf2):
    f32 = np.float32
    # conv1 banded stationaries. Feature (c, j): m = j//16, eo = j%2,
    # e = (j%16)//2, block bi = 2m+eo, partition p = e*16 + c. Padded
    # tap index lp = 5j + k (pad=1 folded in).
    W1full = np.zeros((LP, NB1, 128), f32)
    blk_lp = [[] for _ in range(NB1)]
    for j in range(J1):
        m, eo, e = j // 16, j % 2, (j % 16) // 2
        bi = 2 * m + eo
        for k in range(K1):
            blk_lp[bi].append(5 * j + k)
        for c in range(C1):
            p = e * 16 + c
            for k in range(K1):
                W1full[5 * j + k, bi, p] = w1[c, 0, k]
    mm1 = []  # (bi, w, blob_idx, start, stop)
    w1_mats = []
    for bi in range(NB1):
        lo, hi = min(blk_lp[bi]), max(blk_lp[bi])
        ws = sorted({lo // 128, hi // 128})
        for i, w in enumerate(ws):
            mm1.append((bi, w, len(w1_mats), i == 0, i == len(ws) - 1))
            w1_mats.append(W1full[128 * w:128 * w + 128, bi, :])
    W1blob = np.concatenate(w1_mats, axis=1)  # [128, n1*128]

    # conv2 banded stationaries over pooled features. Pooled feature
    # (c, j'): mp = j'//8, partition q = (j'%8)*16 + c. Output feature
    # (co, jj): mb = jj//4, partition r = (jj%4)*32 + co.
    mm2 = []
    w2_mats = []
    for mb in range(NB2):
        jjs = [jj for jj in range(4 * mb, min(4 * mb + 4, J2))]
        mps = sorted({(3 * jj + k - 1) // 8 for jj in jjs for k in range(K2)
                      if 0 <= 3 * jj + k - 1 < JP})
        for i, mp in enumerate(mps):
            S = np.zeros((128, 128), f32)
            for jj in jjs:
                for k in range(K2):
                    jp = 3 * jj + k - 1
                    if 0 <= jp < JP and jp // 8 == mp:
                        q0 = (jp % 8) * 16
                        for c in range(C1):
                            for co in range(C2):
                                S[q0 + c, (jj - 4 * mb) * 32 + co] = w2[co, c, k]
            mm2.append((mb, mp, len(w2_mats), i == 0, i == len(mps) - 1))
            w2_mats.append(S)
    W2blob = np.concatenate(w2_mats, axis=1)  # [128, n2*128]

    # fc1 stationaries: spk2 partition layout (block mb, partition r) ->
    # wf1 column co*22 + jj.
    WF1 = np.zeros((128, NB2 * 32), f32)
    for mb in range(NB2):
        for jj in range(4 * mb, min(4 * mb + 4, J2)):
            for co in range(C2):
                r = (jj - 4 * mb) * 32 + co
                WF1[r, mb * 32:(mb + 1) * 32] = wf1[:, co * J2 + jj]
    wf2T = np.ascontiguousarray(wf2.T).astype(f32)  # [32, 2]

    b1vec = np.array([b1[p % 16] for p in range(128)], f32)[:, None]
    b2vec = np.array([b2[p % 32] for p in range(128)], f32)[:, None]
    bf1vec = bf1.astype(f32)[:, None]
    bf2vec = bf2.astype(f32)[:, None]
    eye64 = np.eye(64, dtype=f32)
    return dict(W1blob=W1blob, W2blob=W2blob, WF1=WF1, wf2T=wf2T,
                b1vec=b1vec, b2vec=b2vec, bf1vec=bf1vec, bf2vec=bf2vec,
                eye64=eye64, mm1=mm1, mm2=mm2)


def _build_program(host):
    import concourse.bacc as bacc
    import concourse.mybir as mybir
    import concourse.tile as tile

    f32 = mybir.dt.float32
    Alu = mybir.AluOpType
    mm1, mm2 = host["mm1"], host["mm2"]
    n1 = max(e[2] for e in mm1) + 1
    n2 = max(e[2] for e in mm2) + 1

    nc = bacc.Bacc("TRN2", target_bir_lowering=False,
                   debug=False, enable_asserts=False, num_devices=NCORES)

    xp_h = nc.dram_tensor("xp", [BL, T, LP], f32, kind="ExternalInput")
    w1_h = nc.dram_tensor("W1blob", list(host["W1blob"].shape), f32, kind="ExternalInput")
    w2_h = nc.dram_tensor("W2blob", list(host["W2blob"].shape), f32, kind="ExternalInput")
    wf1_h = nc.dram_tensor("WF1", list(host["WF1"].shape), f32, kind="ExternalInput")
    wf2_h = nc.dram_tensor("wf2T", [32, 2], f32, kind="ExternalInput")
    b1_h = nc.dram_tensor("b1vec", [128, 1], f32, kind="ExternalInput")
    b2_h = nc.dram_tensor("b2vec", [128, 1], f32, kind="ExternalInput")
    bf1_h = nc.dram_tensor("bf1vec", [32, 1], f32, kind="ExternalInput")
    bf2_h = nc.dram_tensor("bf2vec", [2, 1], f32, kind="ExternalInput")
    eye_h = nc.dram_tensor("eye64", [64, 64], f32, kind="ExternalInput")
    out_h = nc.dram_tensor("out", [2, BL], f32, kind="ExternalOutput")

    TC = 10  # timesteps per x DMA chunk
    F1 = NB1 * 64            # 1152 conv1/mem1 free size
    FP = NM1 * 64            # 576 pooled free size

    with tile.TileContext(nc, trace_sim=False) as tc:
        with tc.tile_pool(name="w", bufs=1) as wp, \
             tc.tile_pool(name="st", bufs=1) as sp, \
             tc.tile_pool(name="x", bufs=2) as xp_pool, \
             tc.tile_pool(name="xt", bufs=2) as xtp, \
             tc.tile_pool(name="ps1", bufs=1, space="PSUM") as ps1, \
             tc.tile_pool(name="ps2", bufs=1, space="PSUM") as ps2:

            W1t = wp.tile([128, n1 * 128], f32)
            W2t = wp.tile([128, n2 * 128], f32)
            WF1t = wp.tile([128, NB2 * 32], f32)
            wf2t = wp.tile([32, 2], f32)
            b1t = wp.tile([128, 1], f32)
            b2t = wp.tile([128, 1], f32)
            bf1t = wp.tile([32, 1], f32)
            bf2t = wp.tile([2, 1], f32)
            eyet = wp.tile([64, 64], f32)
            for t_, h_ in ((W1t, w1_h), (W2t, w2_h), (WF1t, wf1_h),
                           (wf2t, wf2_h), (b1t, b1_h), (b2t, b2_h),
                           (bf1t, bf1_h), (bf2t, bf2_h), (eyet, eye_h)):
                nc.sync.dma_start(out=t_[:], in_=h_.ap())

            mem1 = sp.tile([128, F1], f32)
            spk1 = sp.tile([128, F1], f32)
            pooled = sp.tile([128, FP], f32)
            mem2 = sp.tile([128, NB2 * 64], f32)
            spk2 = sp.tile([128, NB2 * 64], f32)
            mem3 = sp.tile([32, BL], f32)
            spk3 = sp.tile([32, BL], f32)
            mem4 = sp.tile([2, BL], f32)
            spk4 = sp.tile([2, BL], f32)
            acc = sp.tile([2, BL], f32)
            for t_ in (mem1, spk1, pooled, mem2, spk2, mem3, spk3, mem4,
                       spk4, acc):
                nc.vector.memset(t_[:], 0.0)

            # persistent PSUM tiles
            xT_ps = ps1.tile([128, NW * 64], f32)
            h1a = ps1.tile([128, 512], f32)
            h1b = ps1.tile([128, 512], f32)
            h1c = ps1.tile([128, 128], f32)
            h2 = ps2.tile([128, NB2 * 64], f32)
            f1 = ps2.tile([32, BL], f32)
            f2 = ps2.tile([2, BL], f32)

            def h1slice(bi):
                if bi < 8:
                    return h1a[:, 64 * bi:64 * bi + 64]
                if bi < 16:
                    return h1b[:, 64 * (bi - 8):64 * (bi - 8) + 64]
                return h1c[:, 64 * (bi - 16):64 * (bi - 16) + 64]

            # even/odd views of spk1 for the maxpool
            sp1v = spk1[:].rearrange("p (m eo b) -> p m eo b", eo=2, b=64)
            plv = pooled[:].rearrange("p (m b) -> p m b", b=64)

            xtile = None
            for t in range(T):
                tt = t % TC
                if tt == 0:
                    xtile = xp_pool.tile([64, TC, LP], f32)
                    nc.sync.dma_start(out=xtile[:], in_=xp_h.ap()[:, t:t + TC, :])

                # transpose x_t into [l, b] layout (6 windows of 128)
                xT = xtp.tile([128, NW * 64], f32)
                for w in range(NW):
                    nc.tensor.transpose(
                        xT_ps[:, 64 * w:64 * w + 64],
                        xtile[0:64, tt, 128 * w:128 * w + 128],
                        eyet[:])
                nc.scalar.copy(xT[:], xT_ps[:])

                # conv1 -> h1 psum [feature, b]
                for (bi, w, idx, st, sp_) in mm1:
                    nc.tensor.matmul(
                        h1slice(bi),
                        W1t[:, idx * 128:(idx + 1) * 128],
                        xT[:, 64 * w:64 * w + 64],
                        start=st, stop=sp_)

                # LIF1: mem1 = 0.9*mem1 - spk1_prev + (h1 + b1)
                nc.vector.scalar_tensor_tensor(
                    mem1[:], mem1[:], BETA, spk1[:], Alu.mult, Alu.subtract)
                nc.vector.scalar_tensor_tensor(
                    mem1[:, 0:512], h1a[:], b1t[:], mem1[:, 0:512],
                    Alu.add, Alu.add)
                nc.vector.scalar_tensor_tensor(
                    mem1[:, 512:1024], h1b[:], b1t[:], mem1[:, 512:1024],
                    Alu.add, Alu.add)
                nc.vector.scalar_tensor_tensor(
                    mem1[:, 1024:1152], h1c[:], b1t[:], mem1[:, 1024:1152],
                    Alu.add, Alu.add)
                nc.vector.tensor_scalar(
                    spk1[:], mem1[:], THETA, None, Alu.is_gt)
                # maxpool2: even/odd j are adjacent free-column blocks
                nc.vector.tensor_tensor(
                    plv, sp1v[:, :, 0, :], sp1v[:, :, 1, :], Alu.max)

                # conv2
                for (mb, mp, idx, st, sp_) in mm2:
                    nc.tensor.matmul(
                        h2[:, 64 * mb:64 * mb + 64],
                        W2t[:, idx * 128:(idx + 1) * 128],
                        pooled[:, 64 * mp:64 * mp + 64],
                        start=st, stop=sp_)

                # LIF2
                nc.vector.scalar_tensor_tensor(
                    mem2[:], mem2[:], BETA, spk2[:], Alu.mult, Alu.subtract)
                nc.vector.scalar_tensor_tensor(
                    mem2[:], h2[:], b2t[:], mem2[:], Alu.add, Alu.add)
                nc.vector.tensor_scalar(
                    spk2[:], mem2[:], THETA, None, Alu.is_gt)

                # fc1
                for mb in range(NB2):
                    nc.tensor.matmul(
                        f1[:], WF1t[:, mb * 32:(mb + 1) * 32],
                        spk2[:, 64 * mb:64 * mb + 64],
                        start=(mb == 0), stop=(mb == NB2 - 1))

                # LIF3
                nc.vector.scalar_tensor_tensor(
                    mem3[:], mem3[:], BETA, spk3[:], Alu.mult, Alu.subtract)
                nc.vector.scalar_tensor_tensor(
                    mem3[:], f1[:], bf1t[:], mem3[:], Alu.add, Alu.add)
                nc.vector.tensor_scalar(
                    spk3[:], mem3[:], THETA, None, Alu.is_gt)

                # fc2
                nc.tensor.matmul(f2[:], wf2t[:], spk3[:], start=True, stop=True)

                # LIF4 + spike count accumulation
                nc.vector.scalar_tensor_tensor(
                    mem4[:], mem4[:], BETA, spk4[:], Alu.mult, Alu.subtract)
                nc.vector.scalar_tensor_tensor(
                    mem4[:], f2[:], bf2t[:], mem4[:], Alu.add, Alu.add)
                nc.vector.tensor_scalar(
                    spk4[:], mem4[:], THETA, None, Alu.is_gt)
                nc.vector.tensor_tensor(acc[:], acc[:], spk4[:], Alu.add)

            nc.sync.dma_start(out=out_h.ap(), in_=acc[:])

    nc.compile()
    return nc


def kernel(x, w1, b1, w2, b2, wf1, bf1, wf2, bf2):
    from concourse.bass_utils import run_bass_kernel_spmd

    key = "prog"
    if key not in _CACHE:
        host = _build_host_data(w1, b1, w2, b2, wf1, bf1, wf2, bf2)
        _CACHE["host"] = host
        _CACHE[key] = _build_program(host)
    host = _CACHE["host"]
    nc = _CACHE[key]

    xp = np.zeros((B, T, LP), np.float32)
    xp[:, :, 1:1 + L] = x[:, :, 0, :]

    reps = {k: np.ascontiguousarray(host[k]) for k in
            ("W1blob", "W2blob", "WF1", "wf2T", "b1vec", "b2vec",
             "bf1vec", "bf2vec", "eye64")}
    in_maps = []
    for c in range(NCORES):
        m = {"xp": np.ascontiguousarray(xp[c * BL:(c + 1) * BL])}
        m.update(reps)
        in_maps.append(m)

    res = run_bass_kernel_spmd(nc, in_maps, core_ids=list(range(NCORES)))
    outs = [res.results[c]["out"] for c in range(NCORES)]
    return np.concatenate([o.T for o in outs], axis=0).astype(np.float32)
